# revision 71
# baseline (speedup 1.0000x reference)
"""Trainium2 Bass kernel for the bilinear block classifier.

logits[n, c] = sum_{k,i,j} W[c, k*4096+i*64+j] * head[n, 64k+i] * tail[n, 64k+j] + b[c]
head/tail [4096, 768] fp32, W [97, 49152] fp32, b [97] fp32.

Data-parallel over 8 NeuronCores (512 samples each). All W data is pre-scaled
by 2048 on the host so fp8 tiles stay in e4m3's normal range; the epilogue
rescales by 1/2048 while adding the bias.

Per core the 384 feature chunks ([128 features x 512 samples] slabs of the
blocked outer product) are sourced five ways:
  a2 : PE DoubleRow-fp8 selection matmul reconstructs head rows from packed
       (hi, lo) e4m3 pairs into PSUM; VectorE multiplies straight from PSUM.
  a1 : as a2, but pairs of chunks land in one 2-bank PSUM tile, ScalarE
       evacuates the pair to fp16, VectorE multiplies (fused, free-dim-repeat).
  a1p: as a1 with the multiply on GpSimd instead of VectorE.
  hf : host-quantized product tiles, e4m3 hi+lo per chunk pair, streamed by
       DMA; contraction uses 3 DoubleRow matmuls per pair per sample group
       (hi*Whi + hi*Wlo + lo*Whi; the lo*Wlo term is dropped).
  h1 : "hw" pairs: feature hi tile only (half the DMA bytes) but both W
       terms (2 DoubleRow matmuls: hi*Whi + hi*Wlo); carries only the
       feature-quantization error, so ~2x the chunks fit one error budget.
A-chunks contract via fp16 matmuls [chunk 128x128 stationary] x [W 128x97
moving]; accumulators pack two sample groups per PSUM bank. The host
reassembles [4096, 97] from per-core [512, 97] slabs.
"""

import numpy as np
import ml_dtypes

EMB = 768
BLK = 64
NCLS = 97
NTOT = 4096
NB = 12             # feature blocks of 64
NCORES = 8
NPC = NTOT // NCORES    # 512 samples per core
NM = BLK // 2           # 32 chunks per block (2 i-rows x 64 j each)
WSCALE = 2048.0
F8 = ml_dtypes.float8_e4m3

# per-block species counts; NA1 + NA1P + NA2 + 2*(NHF + NH1) = 32
# graded: A-heavy early (DMA stream is cold), H-heavy late (the last blocks'
# contraction must not wait on stage1 product chains)
NA1 = [4, 4, 4, 4, 4, 4, 4, 4, 4, 4, 4, 0]   # r1 -> ACT evac -> DVE mul (even)
NA1P = [6, 6, 6, 6, 6, 6, 6, 6, 6, 6, 4, 0]  # r1 -> ACT evac -> Pool mul (even)
NA2 = [8, 8, 8, 8, 8, 8, 8, 8, 8, 8, 4, 8]   # r1 pair -> DVE fused PSUM mul (even)
NHF = [0, 0, 0, 0, 0, 0, 0, 0, 0, 0, 0, 0]   # full H pairs (hi+lo C, 3 DR mms)
NH1 = [7, 7, 7, 7, 7, 7, 7, 7, 7, 7, 10, 12]  # hi-C pairs, W hi+lo (2 DR mms)
NA = [a + b + c for a, b, c in zip(NA1, NA1P, NA2)]
for k in range(NB):
    assert NA[k] + 2 * (NHF[k] + NH1[k]) == NM, k
    assert NA1[k] % 2 == 0 and NA1P[k] % 2 == 0 and NA2[k] % 2 == 0

N_WARMUP = 10

_CACHE = {}


def _block_order(k):
    """Per-block contraction order: a2 singles first, then proportional
    interleave of remaining a-chunks with H pairs. Entries: ("a", m) for
    chunk m (one 97-col fp16 W slice), ("hf"|"h1", p) for H pair p (388/194
    fp8 W cols)."""
    na1, na1p, na2 = NA1[k], NA1P[k], NA2[k]
    na = na1 + na1p + na2
    a_rest = list(range(na2, na))
    h_entries = [("hf", p) for p in range(NHF[k])] + [
        ("h1", p) for p in range(NHF[k], NHF[k] + NH1[k])
    ]
    order = [("a", m) for m in range(na2)]
    if k < NB - 1:
        order += h_entries
        order += [("a", m) for m in a_rest]
    else:
        # last block: H tiles are the final DMA arrivals; consume them last
        order += [("a", m) for m in a_rest]
        order += h_entries
    return order


def _w_cols(kind):
    return 97 if kind == "a" else 388


def _h_cols(kind):
    return 2048 if kind == "hf" else 1024


def _block_layout(k):
    """Column offsets into the per-block W16/W8/H streams, keyed by order
    position. Returns (order, w16_off, w8_off, h_off, tot16, tot8, toth)."""
    order = _block_order(k)
    w16_off, w8_off, h_off = {}, {}, {}
    c16 = c8 = ch = 0
    for ei, (kind, idx) in enumerate(order):
        if kind == "a":
            w16_off[ei] = c16
            c16 += 97
        else:
            w8_off[ei] = c8
            c8 += _w_cols(kind)
            h_off[ei] = ch
            ch += _h_cols(kind)
    return order, w16_off, w8_off, h_off, c16, c8, ch


_LAYOUT = [_block_layout(k) for k in range(NB)]
TOT16 = sum(t[4] for t in _LAYOUT)
TOT8 = sum(t[5] for t in _LAYOUT)
TOTH = sum(t[6] for t in _LAYOUT)
O16 = np.cumsum([0] + [t[4] for t in _LAYOUT]).tolist()
O8 = np.cumsum([0] + [t[5] for t in _LAYOUT]).tolist()
OH = np.cumsum([0] + [t[6] for t in _LAYOUT]).tolist()
MAX16 = max(_LAYOUT[k][4] + _LAYOUT[k + 1][4] for k in range(0, NB, 2))
MAX8 = max(_LAYOUT[k][5] + _LAYOUT[k + 1][5] for k in range(0, NB, 2))


def _apair_offsets():
    """Slot offset of each block's A-chunks within its partition group
    (bi = k % 4), plus the max slab size in slots."""
    offs = {}
    group_tot = [0] * 4
    for k in range(NB):
        bi = k % 4
        offs[k] = group_tot[bi]
        group_tot[bi] += NA[k]
    return offs, max(group_tot)


AOFFS, ASLOTS = _apair_offsets()


def _h_tiles(k):
    """Group the block's H order-entries into DMA tiles of <= 4 pairs.
    Returns list of [(ei, kind, cols), ...] per tile, in order."""
    order = _LAYOUT[k][0]
    hs = [(ei, kind) for ei, (kind, idx) in enumerate(order) if kind != "a"]
    tiles = []
    for i in range(0, len(hs), 4):
        grp = [(ei, kind, _h_cols(kind)) for ei, kind in hs[i:i + 4]]
        tiles.append(grp)
    return tiles


def _split_excess_waits(nc, limit=1):
    """walrus in this toolchain rejects instructions carrying more than
    `limit` semaphore waits; split extras into preceding wait-only Drains."""
    import concourse.mybir as mybir

    n_new = 0
    for bb in nc.main_func.blocks:
        new_list = []
        for ins in bb.instructions:
            si = ins.sync_info
            if si is not None and si.on_wait and len(si.on_wait) > limit:
                waits = list(si.on_wait)
                extra, keep = waits[:-limit], waits[-limit:]
                for i in range(0, len(extra), limit):
                    chunk = extra[i : i + limit]
                    n_new += 1
                    d = mybir.InstDrain(
                        name=f"I-waitsplit-{n_new}",
                        engine=ins.engine,
                        ins=[],
                        outs=[],
                        sync_info=mybir.SyncInfo(on_wait=chunk, on_update=[]),
                    )
                    nc.register_instruction(d)
                    new_list.append(d)
                si.on_wait = keep
            new_list.append(ins)
        bb.instructions[:] = new_list
    return n_new


def _build_nc():
    import concourse.bass as bass
    import concourse.mybir as mybir
    import concourse.tile as tile

    dt = mybir.dt
    DR = mybir.MatmulPerfMode.DoubleRow
    nc = bass.Bass()

    b1p8 = nc.dram_tensor(
        "b1p8", [8, 256 + ASLOTS * 1024], dt.float8e4, kind="ExternalInput"
    )
    wt16 = nc.dram_tensor("wt16", [128, TOT16], dt.float16, kind="ExternalInput")
    wt8 = nc.dram_tensor("wt8", [128, TOT8], dt.float8e4, kind="ExternalInput")
    b2d = nc.dram_tensor("b2d", [128, NB * NPC], dt.float16, kind="ExternalInput")
    hbl8 = nc.dram_tensor("hbl8", [128, TOTH], dt.float8e4, kind="ExternalInput")
    bia = nc.dram_tensor("bias128", [128, NCLS], dt.float32, kind="ExternalInput")
    out = nc.dram_tensor("logits_t", [128, 4 * NCLS], dt.float32, kind="ExternalOutput")

    with tile.TileContext(nc) as tc:
        with (
            tc.tile_pool(name="cst", bufs=1) as cst,
            tc.tile_pool(name="wp16", bufs=2) as wp16,
            tc.tile_pool(name="wp8", bufs=2) as wp8,
            tc.tile_pool(name="b2p", bufs=2) as b2p,
            tc.tile_pool(name="hbp", bufs=7) as hbp,
            tc.tile_pool(name="tmpp", bufs=6) as tmpp,
            tc.tile_pool(name="ablp", bufs=2) as ablp,
            tc.tile_pool(name="a2p", bufs=14) as a2p,
            tc.tile_pool(name="psp", bufs=3, space="PSUM") as psp,
            tc.tile_pool(name="pss", bufs=1, space="PSUM") as pss,
            tc.tile_pool(name="accp", bufs=1, space="PSUM") as accp,
        ):
            b1sb = cst.tile([128, 256 + ASLOTS * 1024], dt.float8e4, tag="b1")
            biasb = cst.tile([128, NCLS], dt.float32, tag="bias")
            lgsb = cst.tile([128, 4 * NCLS], dt.float32, tag="logits")

            def load_grouped(dst, src, f):
                """Place src rows (2 per group) at partitions 32*bi. Issued
"""
                for bi in range(4):
                    nc.sync.dma_start(
                        dst[32 * bi : 32 * bi + 2, :], src[2 * bi : 2 * bi + 2, :]
                    )

            # all four 97-col accumulators in one PSUM bank at column
            # offsets 0/128/256/384; start flags fire on s == 0 only
            accT = accp.tile([128, NPC], dt.float32, tag="accT", name="accT")

            def acc_ap(s):
                return accT[:, 128 * s : 128 * s + NCLS]

            blk = {}

            def issue_wb(k, split=False):
                # one DMA each for blocks (k, k+1): b2 first, it gates the
                # multiplies that free r1 PSUM tiles. split=True issues
                # per-block pieces for finer arrival at startup.
                assert k % 2 == 0
                c16 = O16[k + 2] - O16[k]
                c8 = O8[k + 2] - O8[k]
                j16 = O16[k + 1] - O16[k]
                j8 = O8[k + 1] - O8[k]
                b2k = b2p.tile([128, 2 * NPC], dt.float16, tag="b2k", name="b2k")
                wk16 = wp16.tile([128, MAX16], dt.float16, tag="wk16", name="wk16")
                wk8 = wp8.tile([128, MAX8], dt.float8e4, tag="wk8", name="wk8")
                if split:
                    for j in range(2):
                        nc.sync.dma_start(
                            b2k[:, j * NPC : (j + 1) * NPC],
                            b2d[:, (k + j) * NPC : (k + j + 1) * NPC],
                        )
                        s16 = (j16, c16 - j16)[j]
                        s8 = (j8, c8 - j8)[j]
                        nc.sync.dma_start(
                            wk16[:, j * j16 : j * j16 + s16],
                            wt16[:, O16[k + j] : O16[k + j] + s16],
                        )
                        nc.sync.dma_start(
                            wk8[:, j * j8 : j * j8 + s8],
                            wt8[:, O8[k + j] : O8[k + j] + s8],
                        )
                else:
                    nc.sync.dma_start(b2k[:, :], b2d[:, k * NPC : (k + 2) * NPC])
                    nc.sync.dma_start(
                        wk16[:, 0:c16], wt16[:, O16[k] : O16[k] + c16]
                    )
                    nc.sync.dma_start(
                        wk8[:, 0:c8], wt8[:, O8[k] : O8[k] + c8]
                    )
                for j, kk in enumerate((k, k + 1)):
                    blk[kk] = {
                        "b2k": b2k[:, j * NPC : (j + 1) * NPC],
                        "w16": wk16[:, j * j16 : c16],
                        "w8": wk8[:, j * j8 : c8],
                        "htiles": {},
                    }

            def issue_hba(k):
                tiles = _h_tiles(k)
                if not tiles:
                    return
                _issue_htile(k, tiles, 0)

            def issue_hbb(k):
                tiles = _h_tiles(k)
                for ti in range(1, len(tiles)):
                    _issue_htile(k, tiles, ti)

            def _issue_htile(k, tiles, ti):
                grp = tiles[ti]
                cols = sum(c for _, _, c in grp)
                hb = hbp.tile([128, 4 * 1024], dt.float8e4, tag="hb", name="hb")
                base = OH[k] + _LAYOUT[k][3][grp[0][0]]
                nc.sync.dma_start(hb[:, 0:cols], hbl8[:, base : base + cols])
                off = 0
                for ei, kind, c in grp:
                    blk[k]["htiles"][ei] = (hb, off)
                    off += c

            def r1_dr(k, m, out_ap):
                """DoubleRow selection matmul: reconstruct head-row pair
                (2m, 2m+1) from fp8 (hi, lo) into PSUM [128, 512]."""
                bi = k % 4
                b = 32 * bi
                sl = AOFFS[k] + m
                nc.tensor.matmul(
                    out_ap,
                    b1sb[b : b + 2, 0:256].rearrange("p (r f) -> p r f", r=2),
                    b1sb[b : b + 2, 256 + sl * 1024 : 256 + (sl + 1) * 1024].rearrange(
                        "p (r f) -> p r f", r=2
                    ),
                    start=True,
                    stop=True,
                    skip_group_check=True,
                    perf_mode=DR,
                    tile_position=(b, 0),
                )

            def make_stage1(k):
                """Thunks producing the A-chunk product tiles of block k."""
                d = blk[k]
                b2k = d["b2k"]
                na1, na1p, na2 = NA1[k], NA1P[k], NA2[k]
                na = na1 + na1p + na2
                if na1 + na1p:
                    ablb = ablp.tile(
                        [128, max(NA) * NPC], dt.float16, tag="abl", name="ablb"
                    )
                else:
                    ablb = None
                a2bs = {}
                d["abl"] = ablb
                d["a2bs"] = a2bs
                thunks = []

                def mk_a2(m0):
                    def run():
                        pt = psp.tile([128, 2 * NPC], dt.float32, tag="r1p", name="r1p")
                        r1_dr(k, m0, pt[:, 0:NPC])
                        r1_dr(k, m0 + 1, pt[:, NPC : 2 * NPC])
                        a2b = a2p.tile([128, 2 * NPC], dt.float16, tag="a2", name="a2")
                        nc.vector.tensor_mul(
                            a2b[:, :].rearrange("p (r f) -> p r f", r=2),
                            pt[:, :].rearrange("p (r f) -> p r f", r=2),
                            b2k[:, :].unsqueeze(1).to_broadcast([128, 2, NPC]),
                        )
                        a2bs[m0] = (a2b, 0)
                        a2bs[m0 + 1] = (a2b, 1)

                    return run


                def mk_pair(m0, on_pool):
                    def run():
                        pt = psp.tile([128, 2 * NPC], dt.float32, tag="r1p", name="r1p")
                        r1_dr(k, m0, pt[:, 0:NPC])
                        r1_dr(k, m0 + 1, pt[:, NPC : 2 * NPC])
                        tb = tmpp.tile([128, 2 * NPC], dt.float16, tag="tmp", name="tmp")
                        nc.scalar.copy(tb[:, :], pt[:, :])
                        eng = nc.gpsimd if on_pool else nc.vector
                        eng.tensor_mul(
                            ablb[:, m0 * NPC : (m0 + 2) * NPC].rearrange(
                                "p (r f) -> p r f", r=2
                            ),
                            tb[:, :].rearrange("p (r f) -> p r f", r=2),
                            b2k[:, :].unsqueeze(1).to_broadcast([128, 2, NPC]),
                        )

                    return run

                # spread consumers: a2 (DVE-psum), a1 (ACT+DVE), a1p (ACT+Pool)
                # round-robin so no engine's queue bunches up
                specs = (
                    [("a2", 2 * i) for i in range(na2 // 2)],
                    [("a1", na2 + 2 * i, False) for i in range(na1 // 2)],
                    [("a1p", na2 + na1 + 2 * i, True) for i in range(na1p // 2)],
                )
                mixed = []
                for i in range(max(len(s) for s in specs)):
                    for s in specs:
                        if i < len(s):
                            mixed.append(s[i])
                for sp in mixed:
                    if sp[0] == "a2":
                        thunks.append(mk_a2(sp[1]))
                    else:
                        thunks.append(mk_pair(sp[1], sp[2]))
                return thunks

            first_flag = {"v": True}

            def contract(k, ei, kind, idx, last, subs=(0, 1, 2, 3)):
                """Accumulation matmuls for one order entry of block k."""
                d = blk[k]
                order, w16o, w8o, ho = _LAYOUT[k][0], _LAYOUT[k][1], _LAYOUT[k][2], _LAYOUT[k][3]
                first = first_flag["v"]
                if 0 in subs:
                    first_flag["v"] = False
                if kind == "a":
                    m = idx
                    na2 = NA2[k]
                    if m < na2:
                        a2b, half = d["a2bs"][m]
                        src = a2b[:, half * NPC : (half + 1) * NPC]
                    else:
                        src = d["abl"][:, m * NPC : (m + 1) * NPC]
                    co = w16o[ei]
                    for s in subs:
                        nc.tensor.matmul(
                            acc_ap(s),
                            src[:, s * 128 : (s + 1) * 128],
                            d["w16"][:, co : co + NCLS],
                            start=(first and s == 0),
                            stop=(last and s == 3),
                            skip_group_check=True,
                        )
                    return
                # H pair: DR matmuls
                hb, hoff = d["htiles"][ei]
                co = w8o[ei]
                whi = d["w8"][:, co : co + 194].rearrange("p (r n) -> p r n", r=2)
                wlo = d["w8"][:, co + 194 : co + 388].rearrange(
                    "p (r n) -> p r n", r=2
                )
                hi3 = hb[:, hoff : hoff + 1024].rearrange("p (r f) -> p r f", r=2)
                if kind == "hf":
                    lo3 = hb[:, hoff + 1024 : hoff + 2048].rearrange(
                        "p (r f) -> p r f", r=2
                    )
                for s in subs:
                    st = first and s == 0
                    sp = last and s == 3
                    nc.tensor.matmul(
                        acc_ap(s), hi3[:, :, s * 128 : (s + 1) * 128], whi,
                        start=st, stop=False, skip_group_check=True, perf_mode=DR,
                    )
                    if kind == "hf":
                        nc.tensor.matmul(
                            acc_ap(s), hi3[:, :, s * 128 : (s + 1) * 128], wlo,
                            start=False, stop=False, skip_group_check=True,
                            perf_mode=DR,
                        )
                        nc.tensor.matmul(
                            acc_ap(s), lo3[:, :, s * 128 : (s + 1) * 128], whi,
                            start=False, stop=sp, skip_group_check=True, perf_mode=DR,
                        )
                    else:
                        nc.tensor.matmul(
                            acc_ap(s), hi3[:, :, s * 128 : (s + 1) * 128], wlo,
                            start=False, stop=sp, skip_group_check=True, perf_mode=DR,
                        )


            # warmup stationary needs no DMA: memset a small tile so the PE
            # can start ramping its p-state immediately
            wut = cst.tile([128, 128], dt.float16, tag="wut")
            nc.gpsimd.memset(wut[:, :], 1.0)

            # r1 inputs first (they gate the whole stage1 chain), then block
            # 0/1 weights, then the H stream
            load_grouped(b1sb, b1p8, 256 + ASLOTS * 1024)
            issue_wb(0, split=True)
            issue_hba(0)
            issue_hbb(0)
            nc.sync.dma_start(biasb[:, :], bia[:, :])
            issue_hba(1)
            issue_hbb(1)
            issue_wb(2)
            issue_hba(2)

            # interleave warmups with block-0 stage1 so the r1 burst is paced
            # into the PSUM ring as consumers spin up
            s1 = make_stage1(0)
            si0 = 0
            for wi in range(N_WARMUP):
                # warmups target a spare single bank so they never contend
                # for r1 PSUM slots
                wups = pss.tile([128, 128], dt.float32, tag="wu", name="wu")
                nc.tensor.matmul(
                    wups[:, :], wut[:, :], wut[:, :],
                    start=True, stop=True, skip_group_check=True,
                )
                if wi >= 2 and wi % 2 == 0 and si0 < len(s1):
                    s1[si0]()
                    si0 += 1
            for t in s1[si0:]:
                t()

            for k in range(NB):
                if k + 2 < NB:
                    issue_hbb(k + 2)
                if k + 3 < NB:
                    if (k + 3) % 2 == 0:
                        issue_wb(k + 3)
                    issue_hba(k + 3)
                s1 = make_stage1(k + 1) if k + 1 < NB else []
                si = 0
                order = _LAYOUT[k][0]
                ne = len(order)
                if k < NB - 1:
                    for ei, (kind, idx) in enumerate(order):
                        contract(k, ei, kind, idx, False)
                        want = min(len(s1), (ei + 1) * len(s1) // ne)
                        while si < want:
                            s1[si]()
                            si += 1
                    while si < len(s1):
                        s1[si]()
                        si += 1
                else:
                    # last block: finish accumulator bank A first so its
                    # evacuation overlaps the bank-B matmuls
                    for ei, (kind, idx) in enumerate(order):
                        contract(k, ei, kind, idx, ei == ne - 1, subs=(0, 1))
                    nc.vector.scalar_tensor_tensor(
                        lgsb[:, 0 : 2 * NCLS].rearrange("p (r f) -> p r f", r=2),
                        accT[:, 0:256].rearrange("p (r f) -> p r f", r=2)[:, :, 0:NCLS],
                        1.0 / WSCALE,
                        biasb[:, :].unsqueeze(1).to_broadcast([128, 2, NCLS]),
                        op0=mybir.AluOpType.mult,
                        op1=mybir.AluOpType.add,
                    )
                    nc.sync.dma_start(out[:, 0 : 2 * NCLS], lgsb[:, 0 : 2 * NCLS])
                    for ei, (kind, idx) in enumerate(order):
                        contract(k, ei, kind, idx, ei == ne - 1, subs=(2, 3))

            # fused final evacuation: banks 2,3 in one strided STT so the
            # output DMA waits on a single producer
            nc.vector.scalar_tensor_tensor(
                lgsb[:, 2 * NCLS :].rearrange("p (r f) -> p r f", r=2),
                accT[:, 256:].rearrange("p (r f) -> p r f", r=2)[:, :, 0:NCLS],
                1.0 / WSCALE,
                biasb[:, :].unsqueeze(1).to_broadcast([128, 2, NCLS]),
                op0=mybir.AluOpType.mult,
                op1=mybir.AluOpType.add,
            )
            nc.sync.dma_start(out[:, 2 * NCLS :], lgsb[:, 2 * NCLS :])

    _split_excess_waits(nc, limit=1)
    return nc


def _f8(x):
    return x.astype(F8)


def _prep_shared(W, b):
    """Host W layout: per block, contraction-ordered segments.
    wt16 fp16 cols for A-chunks; wt8 fp8 hi/lo cols for H pairs. All x2048."""
    Wr = (np.asarray(W, np.float32) * WSCALE).reshape(NCLS, NB, NM, 2, BLK)
    # Wcols[k][m]: [128, 97] with partition p = d*64+j
    wt16 = np.empty((128, TOT16), np.float16)
    wt8 = np.empty((128, TOT8), F8)
    for k in range(NB):
        order, w16o, w8o, _, _, _, _ = _LAYOUT[k]
        na = NA[k]
        Wk = Wr[:, k]  # [C, NM, 2, 64]
        Wp = Wk.transpose(2, 3, 1, 0).reshape(128, NM, NCLS)  # [p, m, c]
        for ei, (kind, idx) in enumerate(order):
            if kind == "a":
                wt16[:, O16[k] + w16o[ei] : O16[k] + w16o[ei] + 97] = Wp[:, idx].astype(
                    np.float16
                )
            else:
                m1 = na + 2 * idx
                co = O8[k] + w8o[ei]
                whi1 = _f8(Wp[:, m1])
                whi2 = _f8(Wp[:, m1 + 1])
                wt8[:, co : co + 97] = whi1
                wt8[:, co + 97 : co + 194] = whi2
                wlo1 = _f8(Wp[:, m1] - whi1.astype(np.float32))
                wlo2 = _f8(Wp[:, m1 + 1] - whi2.astype(np.float32))
                wt8[:, co + 194 : co + 291] = wlo1
                wt8[:, co + 291 : co + 388] = wlo2
    bias128 = np.ascontiguousarray(
        np.broadcast_to(np.asarray(b, np.float32), (128, NCLS))
    )
    # s2[2bi+d, r*128 + p] = 1 iff p//64 == d (same selection for hi and lo);
    # prepended to each b1p8 row on the host
    s2f = np.zeros((8, 256), np.float32)
    for bi in range(4):
        for r in range(2):
            s2f[2 * bi, r * 128 : r * 128 + 64] = 1.0
            s2f[2 * bi + 1, r * 128 + 64 : r * 128 + 128] = 1.0
    return wt16, wt8, bias128, _f8(s2f)


def _prep_core(head, tail):
    b1T = np.asarray(head, np.float32).T.copy()  # [768, NPC]
    b2T = np.asarray(tail, np.float32).T.reshape(NB, BLK, NPC)

    hhi = _f8(b1T)
    hlo = _f8(b1T - hhi.astype(np.float32))

    # A-route packed head pairs: partition 2bi+d, 256 bytes of selection
    # pattern then slot AOFFS[k]+m at [hi 512 | lo 512] fp8
    b1p8 = np.zeros((8, 256 + ASLOTS * 1024), F8)
    for k in range(NB):
        bi = k % 4
        for m in range(NA[k]):
            sl = 256 + (AOFFS[k] + m) * 1024
            for d in (0, 1):
                row = 64 * k + 2 * m + d
                b1p8[2 * bi + d, sl : sl + NPC] = hhi[row]
                b1p8[2 * bi + d, sl + NPC : sl + 1024] = hlo[row]

    # duplicated tail tile per block: b2d[p, k*512 + n] = t[64k + p%64, n]
    b2dup = np.concatenate([b2T, b2T], axis=1)  # [12, 128, NPC]
    b2d = np.ascontiguousarray(
        b2dup.transpose(1, 0, 2).reshape(128, NB * NPC).astype(np.float16)
    )

    # host-built H product tiles, fp8 hi/lo, pair-packed
    b2f = b2T  # fp32
    hbl = np.empty((128, TOTH), F8)
    for k in range(NB):
        order, _, _, ho, _, _, _ = _LAYOUT[k]
        na = NA[k]
        for ei, (kind, idx) in enumerate(order):
            if kind == "a":
                continue
            base = OH[k] + ho[ei]
            m1 = na + 2 * idx
            for which in (0, 1):
                m = m1 + which
                rows = b1T[[64 * k + 2 * m, 64 * k + 2 * m + 1]]  # [2, NPC]
                prod = (
                    rows[:, None, :] * b2f[k][None, :, :]
                ).reshape(128, NPC)  # p = d*64+j
                phi = _f8(prod)
                hbl[:, base + which * NPC : base + (which + 1) * NPC] = phi
                if kind == "hf":
                    plo = _f8(prod - phi.astype(np.float32))
                    hbl[:, base + 1024 + which * NPC : base + 1024 + (which + 1) * NPC] = plo
    return b1p8, b2d, hbl


def kernel(head_embeddings, tail_embeddings, W, b):
    from concourse.bass_utils import run_bass_kernel_spmd

    assert head_embeddings.shape == (NTOT, EMB), head_embeddings.shape
    assert tail_embeddings.shape == (NTOT, EMB), tail_embeddings.shape
    assert W.shape == (NCLS, EMB * BLK), W.shape

    if "nc" not in _CACHE:
        _CACHE["nc"] = _build_nc()
    nc = _CACHE["nc"]

    wt16, wt8, bias128, s2f = _prep_shared(W, b)
    in_maps = []
    for i in range(NCORES):
        s = slice(i * NPC, (i + 1) * NPC)
        b1p8, b2d, hbl = _prep_core(head_embeddings[s], tail_embeddings[s])
        b1p8[:, 0:256] = s2f
        in_maps.append(
            {
                "b1p8": b1p8,
                "wt16": wt16,
                "wt8": wt8,
                "b2d": b2d,
                "hbl8": hbl,
                "bias128": bias128,
            }
        )

    res = run_bass_kernel_spmd(nc, in_maps, list(range(NCORES)))
    _CACHE["last_results"] = res
    parts = []
    for i in range(NCORES):
        lg = res.results[i]["logits_t"]  # [128, 4*97]
        parts.append(lg.reshape(128, 4, NCLS).transpose(1, 0, 2).reshape(NPC, NCLS))
    return np.concatenate(parts, axis=0).astype(np.float32)
